# revision 40
# baseline (speedup 1.0000x reference)
"""Trainium2 Bass kernel for nn_BaseViewTransform (BEVFusion bev_pool / segment-mean).

Pipeline:
  Host (index plane + sharding, derived from the 5 small input matrices):
    - compute per-point voxel/segment ids exactly as the reference (float32
      geometry, truncation toward zero)
    - sort kept points by segment id; shard = contiguous sorted range per core;
      materialize each core's shard as a contiguous fp8(e3m4) point buffer
    - group points into 512-point "strips" (4 chunks of 128); each strip has
      <= 32 distinct segments (measured max 22); per-point `rel` = slot index
      of its segment within the strip
  Device (single SPMD program, all heavy compute):
    - streams the point shard contiguously (~1 MB HWDGE transfers)
    - one-hot [128, 32] built on DVE via per-slot tensor_scalar is_equal
      (dense step-1 APs -> 4x perf mode)
    - per chunk one matmul: onehot[128,32] (stationary, cheap LDWEIGHTS)
      x feats[128,80] fp8 (streaming); 4 chunks of a strip accumulate into the
      strip's [32, 80] PSUM region (start/stop accumulation group)
    - strips are laid out 4(v) x 6(h) per PSUM bank, rotating over 6 banks so
      consecutive strips hit different banks AND different col-groups
      (col-tiled matmul overlap; bank separation keeps has_written safe)
    - full banks are copied [128, 480] f32->bf16 on ACT and DMA'd out
  Host: per-strip slot sums -> segment sums, divide by counts, scatter into
  the dense [1, 80, 360, 360] BEV grid (empty voxels stay 0 like the ref).
"""

import os
import numpy as np
import ml_dtypes

# ---------------- problem constants (hardcoded per task rules) ----------------
IMAGE_SIZE = (256, 704)
FEATURE_SIZE = (32, 88)
XBOUND = (-54.0, 54.0, 0.3)
YBOUND = (-54.0, 54.0, 0.3)
ZBOUND = (-10.0, 10.0, 20.0)
DBOUND = (1.0, 60.0, 0.5)
C_OUT = 80
NX = (360, 360, 1)
NSEG = NX[2] * NX[0] * NX[1]  # 129600
DX = np.array([XBOUND[2], YBOUND[2], ZBOUND[2]], np.float32)
BX = np.array([XBOUND[0] + XBOUND[2] / 2.0,
               YBOUND[0] + YBOUND[2] / 2.0,
               ZBOUND[0] + ZBOUND[2] / 2.0], np.float32)

NCORES = 8
P = 128          # points per chunk (= matmul contraction dim)
WIN = 32         # strip slot stride in psum rows (32-aligned col groups)
WGEN = 24        # one-hot columns actually generated (max span measured 22)
SPC = 4          # chunks per strip (512 points)
NB = 8           # psum banks in rotation
SPB = 24         # strips per bank fill (4 v-slots x 6 h-slots)
HCOL = 6         # h slots per bank (6 x 80 = 480 cols)
GG = 96          # chunks per feat-DMA group (96*80 fp8 = ~983KB)
G0 = 16          # first feat/one-hot group size (fast pipeline start)
OH_G = 208       # chunks per one-hot gen group

USE_FP8 = os.environ.get("KBF16", "0") != "1"
# zero-init psum banks before use: required for CoreSim (it rejects reads of
# uninitialized PSUM); on hardware the unwritten slots are garbage the host
# never reads, so skip the extra matmuls there.
SIM_SAFE = os.environ.get("KSIM", "0") == "1"


def _frustum():
    iH, iW = IMAGE_SIZE
    fH, fW = FEATURE_SIZE
    ds = np.arange(DBOUND[0], DBOUND[1], DBOUND[2], dtype=np.float32)
    xs = np.linspace(0.0, iW - 1.0, fW, dtype=np.float32)
    ys = np.linspace(0.0, iH - 1.0, fH, dtype=np.float32)
    return np.stack(np.broadcast_arrays(
        xs[None, None, :], ys[None, :, None], ds[:, None, None]), -1
    ).astype(np.float32)  # [D, fH, fW, 3]


def _segments(camera_intrinsics, camera2lidar, img_aug_matrix, lidar_aug_matrix):
    """Replicates reference get_geometry + voxelization in numpy float32.
    Returns (seg[Np] int64, kept[Np] bool)."""
    intr = np.asarray(camera_intrinsics, np.float32)
    c2l = np.asarray(camera2lidar, np.float32)
    img_aug = np.asarray(img_aug_matrix, np.float32)
    lidar_aug = np.asarray(lidar_aug_matrix, np.float32)

    intrins = intr[..., :3, :3]
    post_rots = img_aug[..., :3, :3]
    post_trans = img_aug[..., :3, 3]
    rots = c2l[..., :3, :3]
    trans = c2l[..., :3, 3]
    er = lidar_aug[..., :3, :3]
    et = lidar_aug[..., :3, 3]

    f = _frustum()
    pts = f[None, None] - post_trans[:, :, None, None, None, :]
    ipr = np.linalg.inv(post_rots.astype(np.float64)).astype(np.float32)
    pts = np.einsum('bnij,bndhwj->bndhwi', ipr, pts).astype(np.float32)
    pts = np.concatenate([pts[..., :2] * pts[..., 2:3], pts[..., 2:3]], -1)
    iintr = np.linalg.inv(intrins.astype(np.float64)).astype(np.float32)
    comb = np.einsum('bnij,bnjk->bnik', rots, iintr).astype(np.float32)
    pts = (np.einsum('bnij,bndhwj->bndhwi', comb, pts)
           + trans[:, :, None, None, None, :]).astype(np.float32)
    pts = (np.einsum('bij,bndhwj->bndhwi', er, pts)
           + et[:, None, None, None, None, :]).astype(np.float32)

    Np = pts.size // 3
    geom = ((pts - (BX - DX / 2.0)) / DX).astype(np.int32).reshape(Np, 3)
    kept = ((geom[:, 0] >= 0) & (geom[:, 0] < NX[0])
            & (geom[:, 1] >= 0) & (geom[:, 1] < NX[1])
            & (geom[:, 2] >= 0) & (geom[:, 2] < NX[2]))
    seg = (geom[:, 2].astype(np.int64) * (NX[0] * NX[1])
           + geom[:, 0].astype(np.int64) * NX[1]
           + geom[:, 1].astype(np.int64))
    return seg, kept


def _plan(seg, kept):
    """Sort kept points, shard across cores, strip-pack.

    Per core: rows (padded point ids, [nchunk, P]), rel ([nchunk, P], strip
    slot per point, -1 pad), slot_seg ([nstrip, WIN]), span ([nstrip]).
    """
    kidx = np.nonzero(kept)[0].astype(np.int64)
    segk = seg[kidx]
    order = np.argsort(segk, kind='stable')
    rows_sorted = kidx[order]
    seg_sorted = segk[order]
    counts = np.bincount(seg_sorted, minlength=NSEG)

    nk = len(rows_sorted)
    bounds = [int(round(nk * k / NCORES)) for k in range(NCORES + 1)]
    SP = P * SPC  # points per strip

    nstrip = max((bounds[k + 1] - bounds[k] + SP - 1) // SP
                 for k in range(NCORES))
    nchunk = nstrip * SPC

    rows_all = np.zeros((NCORES, nchunk, P), np.int64)
    rel_all = np.full((NCORES, nchunk, P), -1, np.int32)
    slot_seg = np.zeros((NCORES, nstrip, WIN), np.int64)
    span_all = np.zeros((NCORES, nstrip), np.int32)
    maxspan = 0
    for k in range(NCORES):
        lo, hi = bounds[k], bounds[k + 1]
        n = hi - lo
        s = seg_sorted[lo:hi]
        strip_id = np.arange(n) // SP
        new = np.r_[True, np.diff(s) != 0]
        new[::SP] = True
        cum = np.cumsum(new) - 1                        # distinct run index
        first = cum[::SP]                               # per strip
        rel = (cum - first[strip_id]).astype(np.int32)
        assert rel.max() < WIN, f"strip span {rel.max() + 1} > {WIN}"
        maxspan = max(maxspan, int(rel.max()) + 1)
        ns = (n + SP - 1) // SP
        rows_all[k, :, :].reshape(-1)[:n] = rows_sorted[lo:hi]
        rel_all[k, :, :].reshape(-1)[:n] = rel
        slot_seg[k, strip_id, rel] = s
        # span per strip = rel of last point in strip + 1
        last = np.minimum(np.arange(1, ns + 1) * SP, n) - 1
        span_all[k, :ns] = rel[last] + 1
    return dict(nstrip=nstrip, nchunk=nchunk, rows=rows_all, rel=rel_all,
                slot_seg=slot_seg, span=span_all, counts=counts,
                maxspan=maxspan)


# ---------------- device program ----------------
_COMPILED = {}


def _group_starts(nchunk, first, step):
    starts = [0]
    s = first
    while s < nchunk:
        starts.append(s)
        s += step
    return starts


def _build_program(nchunk, nstrip, wgen):
    import concourse.tile as tile
    import concourse.bass as bass
    from concourse import bacc, mybir

    key = (nchunk, nstrip, wgen, USE_FP8, SIM_SAFE)
    if key in _COMPILED:
        return _COMPILED[key]

    fdt = mybir.dt.float8e3 if USE_FP8 else mybir.dt.bfloat16
    bf = mybir.dt.bfloat16
    nfill = (nstrip + NB * SPB - 1) // (NB * SPB)
    nfb = nfill * NB

    nc = bacc.Bacc("TRN2", target_bir_lowering=False, debug=False,
                   enable_asserts=False, num_devices=NCORES)
    pts = nc.dram_tensor("pts", [P, nchunk * C_OUT], fdt,
                         kind="ExternalInput").ap()
    rel = nc.dram_tensor("rel", [P, nchunk], bf, kind="ExternalInput").ap()
    wout = nc.dram_tensor("wout", [nfb, P, HCOL * C_OUT], bf,
                          kind="ExternalOutput").ap()

    def strip_geom(i):
        f, r = divmod(i, NB * SPB)
        b = r % NB
        j = r // NB
        return f, b, (j + b) % 4, j // 4

    # last strip index + max h column used for every (fill, bank)
    bank_last = {}
    bank_maxh = {}
    fill_last = {}
    for i in range(nstrip):
        f, b, v, h = strip_geom(i)
        bank_last[(f, b)] = i
        bank_maxh[(f, b)] = max(bank_maxh.get((f, b), 0), h)
        fill_last[f] = i
    fill_w = {}
    for (f, b), mh in bank_maxh.items():
        fill_w[f] = max(fill_w.get(f, 0), C_OUT * (mh + 1))

    # two small leading groups (land in parallel on the two rings) then GG
    feat_starts = [0, 32]
    s = 64
    while s < nchunk:
        feat_starts.append(s)
        s += GG
    oh_starts = _group_starts(nchunk, G0, OH_G)
    nquad = (nstrip + 3) // 4

    with tile.TileContext(nc) as tc:
        with tc.tile_pool(name="const", bufs=1) as constp, \
             tc.tile_pool(name="feat", bufs=8) as featp, \
             tc.tile_pool(name="oh", bufs=3) as ohp, \
             tc.tile_pool(name="rel", bufs=3) as relp, \
             tc.tile_pool(name="stage", bufs=3) as stagep, \
             tc.tile_pool(name="psum", bufs=8, space="PSUM") as psump:
            zeros_t = constp.tile([P, HCOL * C_OUT], bf)
            if SIM_SAFE:
                nc.vector.memset(zeros_t[:], 0.0)

            # rel arrives in per-oh-group pieces (53KB) so it never delays
            # a ~1MB feat group on either HWDGE ring; piece k is prefetched
            # one oh-group ahead of its use.
            rel_tiles = {}

            def rel_piece(k):
                if k in rel_tiles or k >= len(oh_starts):
                    return
                g0 = oh_starts[k]
                g1 = (oh_starts[k + 1] if k + 1 < len(oh_starts) else nchunk)
                rt = relp.tile([P, OH_G], bf, name="relt")
                eng = nc.sync if k % 2 == 0 else nc.scalar
                eng.dma_start(out=rt[:, :g1 - g0], in_=rel[:, g0:g1])
                rel_tiles[k] = rt

            rel_piece(0)
            rel_piece(1)
            # group-0 one-hot in one broadcast tensor_tensor (cheaper than
            # wgen tiny tensor_scalar ops): oh0[p, t, s] = (iota[s]==rel[p,t])
            iota_t = constp.tile([P, wgen], bf)
            nc.gpsimd.iota(out=iota_t[:], pattern=[[1, wgen]], base=0,
                           channel_multiplier=0,
                           allow_small_or_imprecise_dtypes=True)
            oh0_t = constp.tile([P, G0, wgen], bf)
            i_ap = iota_t[:]
            r_ap = rel_tiles[0][:, :G0]
            nc.vector.tensor_tensor(
                out=oh0_t[:],
                in0=bass.AP(i_ap.tensor, i_ap.offset,
                            [i_ap.ap[0], [0, G0], i_ap.ap[1]]),
                in1=bass.AP(r_ap.tensor, r_ap.offset,
                            list(r_ap.ap) + [[0, wgen]]),
                op=mybir.AluOpType.is_equal)

            bank_tile = {}
            fill_tile = {}
            f_t = oh_t = None
            ft0 = oh0 = 0
            fidx = oidx = 0
            ncopy = 0
            for q in range(nquad):
                strips = [4 * q + d for d in range(4) if 4 * q + d < nstrip]
                t0q = 16 * q

                if fidx < len(feat_starts) and feat_starts[fidx] == t0q:
                    g0 = feat_starts[fidx]
                    g1 = (feat_starts[fidx + 1]
                          if fidx + 1 < len(feat_starts) else nchunk)
                    ng = min(g1, nchunk) - g0
                    f_t = featp.tile([P, GG, C_OUT], fdt, name="ft")
                    # alternate the two HWDGE rings so consecutive group
                    # DMAs overlap instead of serializing FIFO on one ring
                    deng = nc.sync if fidx % 2 == 0 else nc.scalar
                    deng.dma_start(
                        out=f_t[:, :ng],
                        in_=pts[:, g0 * C_OUT:(g0 + ng) * C_OUT].rearrange(
                            "p (t d) -> p t d", d=C_OUT))
                    ft0 = g0
                    fidx += 1
                if oidx < len(oh_starts) and oh_starts[oidx] == t0q:
                    rel_piece(oidx)
                    rel_piece(oidx + 1)  # prefetch the next piece
                    g0 = oh_starts[oidx]
                    g1 = (oh_starts[oidx + 1]
                          if oidx + 1 < len(oh_starts) else nchunk)
                    ng = min(g1, nchunk) - g0
                    if oidx > 0:
                        oh_t = ohp.tile([P, wgen, OH_G], bf, name="oh")
                        rsrc = rel_tiles[oidx][:, :ng]
                        for s in range(wgen):
                            nc.vector.tensor_scalar(
                                out=oh_t[:, s, :ng],
                                in0=rsrc,
                                scalar1=float(s), scalar2=None,
                                op0=mybir.AluOpType.is_equal)
                        oh0 = g0
                    oidx += 1

                geoms = {}
                for i in strips:
                    geoms[i] = strip_geom(i)
                    f, b, v, h = geoms[i]
                    if (f, b) not in bank_tile:
                        ps = psump.tile([P, HCOL * C_OUT], mybir.dt.float32,
                                        name="psbank")
                        bank_tile[(f, b)] = ps
                        if SIM_SAFE:
                            # zero-fill the bank (zero weights x anything) so
                            # rows/cols no strip writes are defined
                            w = fill_w[f]
                            nc.tensor.matmul(
                                out=ps[:, :w], lhsT=zeros_t[:, :P],
                                rhs=zeros_t[:, :w], start=True, stop=True,
                                skip_group_check=True)
                    if f not in fill_tile:
                        fill_tile[f] = stagep.tile(
                            [P, NB, HCOL * C_OUT], bf, name="fillst")
                # consecutive strips land in different PE col-groups (the
                # diagonal v assignment) -> LDW pull-ahead at strip edges
                for i in strips:
                    f, b, v, h = geoms[i]
                    for c in range(SPC):
                        t = SPC * i + c
                        lhsT = (oh0_t[:, t, :] if t < G0
                                else oh_t[:, :, t - oh0])
                        nc.tensor.matmul(
                            out=bank_tile[(f, b)][
                                32 * v:32 * v + wgen,
                                C_OUT * h:C_OUT * (h + 1)],
                            lhsT=lhsT,
                            rhs=f_t[:, t - ft0],
                            start=(c == 0),
                            stop=(c == SPC - 1),
                            tile_position=(0, 32 * v),
                            skip_group_check=True,
                        )
                for i in strips:
                    f, b, v, h = geoms[i]
                    w = fill_w[f]
                    if bank_last[(f, b)] == i:
                        st = fill_tile[f]
                        if ncopy % 2 == 0:
                            nc.scalar.copy(out=st[:, b, :w],
                                           in_=bank_tile[(f, b)][:, :w])
                        else:
                            nc.vector.tensor_copy(
                                out=st[:, b, :w],
                                in_=bank_tile[(f, b)][:, :w])
                        ncopy += 1
                    if fill_last[f] == i:
                        deng = nc.sync if f % 2 == 0 else nc.scalar
                        deng.dma_start(
                            out=wout[f * NB:(f + 1) * NB, :, :w].rearrange(
                                "b p c -> p b c"),
                            in_=fill_tile[f][:, :, :w])

    nc.compile()
    _COMPILED[key] = nc
    return nc


def _run_on_hw(nc, in_maps, trace=False):
    from concourse.bass_utils import run_bass_kernel_spmd
    from concourse.bass_interp import get_hw_module

    if trace:
        try:
            import ntff_hook
            ntff_hook.install()
        except Exception:
            pass
    hw_m = get_hw_module(nc.m)
    old_m = nc.m
    nc.m = hw_m
    try:
        res = run_bass_kernel_spmd(
            nc, in_maps, core_ids=list(range(NCORES)), trace=trace,
        )
    finally:
        nc.m = old_m
    return res


def kernel(cam_feats, camera_intrinsics, camera2lidar, img_aug_matrix,
           lidar_aug_matrix, _trace=False, _return_results=False):
    cam = np.ascontiguousarray(np.asarray(cam_feats, np.float32))
    Npts = cam.size // C_OUT
    fnp = ml_dtypes.float8_e3m4 if USE_FP8 else ml_dtypes.bfloat16
    cam_q = cam.reshape(Npts, C_OUT).astype(fnp)

    seg, kept = _segments(camera_intrinsics, camera2lidar,
                          img_aug_matrix, lidar_aug_matrix)
    plan = _plan(seg, kept)
    nchunk, nstrip = plan['nchunk'], plan['nstrip']

    in_maps = []
    for k in range(NCORES):
        shard = cam_q[plan['rows'][k].reshape(-1)]
        shard = shard.reshape(nchunk, P, C_OUT).transpose(1, 0, 2)
        shard = np.ascontiguousarray(shard).reshape(P, nchunk * C_OUT)
        relk = np.ascontiguousarray(
            plan['rel'][k].T.astype(np.float32)).astype(ml_dtypes.bfloat16)
        in_maps.append(dict(pts=shard, rel=relk))

    wgen = WGEN if plan['maxspan'] <= WGEN else WIN
    nc = _build_program(nchunk, nstrip, wgen)
    res = _run_on_hw(nc, in_maps, trace=_trace)

    # ---------------- host assembly ----------------
    vals = np.stack([np.asarray(r['wout']).astype(np.float32)
                     for r in res.results])  # [NCORES, nfb, 128, 480]

    # strip i -> wout[f*NB + b][32v:32v+32, 80h:80(h+1)]
    i_all = np.arange(nstrip)
    f_i, r_i = np.divmod(i_all, NB * SPB)
    b_i = r_i % NB
    j_i = r_i // NB
    v_i = (j_i + b_i) % 4
    h_i = j_i // 4
    fb = f_i * NB + b_i
    # gather each strip's [WIN, C_OUT] block
    rows = (32 * v_i)[:, None] + np.arange(WIN)[None, :]      # [nstrip, WIN]
    cols = (C_OUT * h_i)[:, None] + np.arange(C_OUT)[None, :]  # [nstrip, C_OUT]
    strip_vals = vals[:, fb[:, None, None], rows[:, :, None], cols[:, None, :]]
    # -> [NCORES, nstrip, WIN, C_OUT]

    segs = plan['slot_seg']
    valid = (np.arange(WIN)[None, None, :] < plan['span'][:, :, None])
    s_all = segs.reshape(NCORES, -1)[valid.reshape(NCORES, -1)]
    v_all = strip_vals.reshape(NCORES, nstrip * WIN, C_OUT)[
        valid.reshape(NCORES, -1)]
    o2 = np.argsort(s_all, kind='stable')
    s2 = s_all[o2]
    v2 = v_all[o2]
    acc = np.zeros((NSEG, C_OUT), np.float32)
    if len(s2):
        starts = np.r_[0, np.flatnonzero(np.diff(s2)) + 1]
        sums = np.add.reduceat(v2, starts, axis=0)
        useg = s2[starts]
        acc[useg] = sums / np.maximum(plan['counts'][useg], 1)[:, None]

    out = acc.reshape(NX[2], NX[0], NX[1], C_OUT).transpose(0, 3, 1, 2)
    out = out.reshape(1, NX[2] * C_OUT, NX[0], NX[1]).astype(np.float32)
    if _return_results:
        return out, res
    return out


# revision 41
# speedup vs baseline: 1.0940x; 1.0940x over previous
"""Trainium2 Bass kernel for nn_BaseViewTransform (BEVFusion bev_pool / segment-mean).

Pipeline:
  Host (index plane + sharding, derived from the 5 small input matrices):
    - compute per-point voxel/segment ids exactly as the reference (float32
      geometry, truncation toward zero)
    - sort kept points by segment id; shard = contiguous sorted range per core;
      materialize each core's shard as a contiguous fp8(e3m4) point buffer
    - group points into 512-point "strips" (4 chunks of 128); each strip has
      <= 32 distinct segments (measured max 22); per-point `rel` = slot index
      of its segment within the strip
  Device (single SPMD program, all heavy compute):
    - streams the point shard contiguously (~1 MB HWDGE transfers)
    - one-hot [128, 32] built on DVE via per-slot tensor_scalar is_equal
      (dense step-1 APs -> 4x perf mode)
    - per chunk one matmul: onehot[128,32] (stationary, cheap LDWEIGHTS)
      x feats[128,80] fp8 (streaming); 4 chunks of a strip accumulate into the
      strip's [32, 80] PSUM region (start/stop accumulation group)
    - strips are laid out 4(v) x 6(h) per PSUM bank, rotating over 6 banks so
      consecutive strips hit different banks AND different col-groups
      (col-tiled matmul overlap; bank separation keeps has_written safe)
    - full banks are copied [128, 480] f32->bf16 on ACT and DMA'd out
  Host: per-strip slot sums -> segment sums, divide by counts, scatter into
  the dense [1, 80, 360, 360] BEV grid (empty voxels stay 0 like the ref).
"""

import os
import numpy as np
import ml_dtypes

# ---------------- problem constants (hardcoded per task rules) ----------------
IMAGE_SIZE = (256, 704)
FEATURE_SIZE = (32, 88)
XBOUND = (-54.0, 54.0, 0.3)
YBOUND = (-54.0, 54.0, 0.3)
ZBOUND = (-10.0, 10.0, 20.0)
DBOUND = (1.0, 60.0, 0.5)
C_OUT = 80
NX = (360, 360, 1)
NSEG = NX[2] * NX[0] * NX[1]  # 129600
DX = np.array([XBOUND[2], YBOUND[2], ZBOUND[2]], np.float32)
BX = np.array([XBOUND[0] + XBOUND[2] / 2.0,
               YBOUND[0] + YBOUND[2] / 2.0,
               ZBOUND[0] + ZBOUND[2] / 2.0], np.float32)

NCORES = 8
P = 128          # points per chunk (= matmul contraction dim)
WIN = 32         # strip slot stride in psum rows (32-aligned col groups)
WGEN = 24        # one-hot columns actually generated (max span measured 22)
SPC = 4          # chunks per strip (512 points)
NB = 8           # psum banks in rotation
SPB = 24         # strips per bank fill (4 v-slots x 6 h-slots)
HCOL = 6         # h slots per bank (6 x 80 = 480 cols)
GG = 96          # chunks per feat-DMA group (96*80 fp8 = ~983KB)
G0 = 16          # first feat/one-hot group size (fast pipeline start)
OH_G = 208       # chunks per one-hot gen group

USE_FP8 = os.environ.get("KBF16", "0") != "1"
# zero-init psum banks before use: required for CoreSim (it rejects reads of
# uninitialized PSUM); on hardware the unwritten slots are garbage the host
# never reads, so skip the extra matmuls there.
SIM_SAFE = os.environ.get("KSIM", "0") == "1"


def _frustum():
    iH, iW = IMAGE_SIZE
    fH, fW = FEATURE_SIZE
    ds = np.arange(DBOUND[0], DBOUND[1], DBOUND[2], dtype=np.float32)
    xs = np.linspace(0.0, iW - 1.0, fW, dtype=np.float32)
    ys = np.linspace(0.0, iH - 1.0, fH, dtype=np.float32)
    return np.stack(np.broadcast_arrays(
        xs[None, None, :], ys[None, :, None], ds[:, None, None]), -1
    ).astype(np.float32)  # [D, fH, fW, 3]


def _segments(camera_intrinsics, camera2lidar, img_aug_matrix, lidar_aug_matrix):
    """Replicates reference get_geometry + voxelization in numpy float32.
    Returns (seg[Np] int64, kept[Np] bool)."""
    intr = np.asarray(camera_intrinsics, np.float32)
    c2l = np.asarray(camera2lidar, np.float32)
    img_aug = np.asarray(img_aug_matrix, np.float32)
    lidar_aug = np.asarray(lidar_aug_matrix, np.float32)

    intrins = intr[..., :3, :3]
    post_rots = img_aug[..., :3, :3]
    post_trans = img_aug[..., :3, 3]
    rots = c2l[..., :3, :3]
    trans = c2l[..., :3, 3]
    er = lidar_aug[..., :3, :3]
    et = lidar_aug[..., :3, 3]

    f = _frustum()
    pts = f[None, None] - post_trans[:, :, None, None, None, :]
    ipr = np.linalg.inv(post_rots.astype(np.float64)).astype(np.float32)
    pts = np.einsum('bnij,bndhwj->bndhwi', ipr, pts).astype(np.float32)
    pts = np.concatenate([pts[..., :2] * pts[..., 2:3], pts[..., 2:3]], -1)
    iintr = np.linalg.inv(intrins.astype(np.float64)).astype(np.float32)
    comb = np.einsum('bnij,bnjk->bnik', rots, iintr).astype(np.float32)
    pts = (np.einsum('bnij,bndhwj->bndhwi', comb, pts)
           + trans[:, :, None, None, None, :]).astype(np.float32)
    pts = (np.einsum('bij,bndhwj->bndhwi', er, pts)
           + et[:, None, None, None, None, :]).astype(np.float32)

    Np = pts.size // 3
    geom = ((pts - (BX - DX / 2.0)) / DX).astype(np.int32).reshape(Np, 3)
    kept = ((geom[:, 0] >= 0) & (geom[:, 0] < NX[0])
            & (geom[:, 1] >= 0) & (geom[:, 1] < NX[1])
            & (geom[:, 2] >= 0) & (geom[:, 2] < NX[2]))
    seg = (geom[:, 2].astype(np.int64) * (NX[0] * NX[1])
           + geom[:, 0].astype(np.int64) * NX[1]
           + geom[:, 1].astype(np.int64))
    return seg, kept


def _plan(seg, kept):
    """Sort kept points, shard across cores, strip-pack.

    Per core: rows (padded point ids, [nchunk, P]), rel ([nchunk, P], strip
    slot per point, -1 pad), slot_seg ([nstrip, WIN]), span ([nstrip]).
    """
    kidx = np.nonzero(kept)[0].astype(np.int64)
    segk = seg[kidx]
    order = np.argsort(segk, kind='stable')
    rows_sorted = kidx[order]
    seg_sorted = segk[order]
    counts = np.bincount(seg_sorted, minlength=NSEG)

    nk = len(rows_sorted)
    bounds = [int(round(nk * k / NCORES)) for k in range(NCORES + 1)]
    SP = P * SPC  # points per strip

    nstrip = max((bounds[k + 1] - bounds[k] + SP - 1) // SP
                 for k in range(NCORES))
    nchunk = nstrip * SPC

    rows_all = np.zeros((NCORES, nchunk, P), np.int64)
    rel_all = np.full((NCORES, nchunk, P), -1, np.int32)
    slot_seg = np.zeros((NCORES, nstrip, WIN), np.int64)
    span_all = np.zeros((NCORES, nstrip), np.int32)
    maxspan = 0
    for k in range(NCORES):
        lo, hi = bounds[k], bounds[k + 1]
        n = hi - lo
        s = seg_sorted[lo:hi]
        strip_id = np.arange(n) // SP
        new = np.r_[True, np.diff(s) != 0]
        new[::SP] = True
        cum = np.cumsum(new) - 1                        # distinct run index
        first = cum[::SP]                               # per strip
        rel = (cum - first[strip_id]).astype(np.int32)
        assert rel.max() < WIN, f"strip span {rel.max() + 1} > {WIN}"
        maxspan = max(maxspan, int(rel.max()) + 1)
        ns = (n + SP - 1) // SP
        rows_all[k, :, :].reshape(-1)[:n] = rows_sorted[lo:hi]
        rel_all[k, :, :].reshape(-1)[:n] = rel
        slot_seg[k, strip_id, rel] = s
        # span per strip = rel of last point in strip + 1
        last = np.minimum(np.arange(1, ns + 1) * SP, n) - 1
        span_all[k, :ns] = rel[last] + 1
    return dict(nstrip=nstrip, nchunk=nchunk, rows=rows_all, rel=rel_all,
                slot_seg=slot_seg, span=span_all, counts=counts,
                maxspan=maxspan)


# ---------------- device program ----------------
_COMPILED = {}


def _group_starts(nchunk, first, step):
    starts = [0]
    s = first
    while s < nchunk:
        starts.append(s)
        s += step
    return starts


def _build_program(nchunk, nstrip, wgen):
    import concourse.tile as tile
    import concourse.bass as bass
    from concourse import bacc, mybir

    key = (nchunk, nstrip, wgen, USE_FP8, SIM_SAFE)
    if key in _COMPILED:
        return _COMPILED[key]

    fdt = mybir.dt.float8e3 if USE_FP8 else mybir.dt.bfloat16
    bf = mybir.dt.bfloat16
    nfill = (nstrip + NB * SPB - 1) // (NB * SPB)
    nfb = nfill * NB

    nc = bacc.Bacc("TRN2", target_bir_lowering=False, debug=False,
                   enable_asserts=False, num_devices=NCORES)
    pts = nc.dram_tensor("pts", [P, nchunk * C_OUT], fdt,
                         kind="ExternalInput").ap()
    rel = nc.dram_tensor("rel", [P, nchunk], bf, kind="ExternalInput").ap()
    wout = nc.dram_tensor("wout", [nfb, P, HCOL * C_OUT], bf,
                          kind="ExternalOutput").ap()

    def strip_geom(i):
        f, r = divmod(i, NB * SPB)
        b = r % NB
        j = r // NB
        return f, b, (j + b) % 4, j // 4

    # last strip index + max h column used for every (fill, bank)
    bank_last = {}
    bank_maxh = {}
    fill_last = {}
    for i in range(nstrip):
        f, b, v, h = strip_geom(i)
        bank_last[(f, b)] = i
        bank_maxh[(f, b)] = max(bank_maxh.get((f, b), 0), h)
        fill_last[f] = i
    fill_w = {}
    for (f, b), mh in bank_maxh.items():
        fill_w[f] = max(fill_w.get(f, 0), C_OUT * (mh + 1))

    feat_starts = _group_starts(nchunk, G0, GG)
    oh_starts = _group_starts(nchunk, G0, OH_G)
    nquad = (nstrip + 3) // 4

    with tile.TileContext(nc) as tc:
        with tc.tile_pool(name="const", bufs=1) as constp, \
             tc.tile_pool(name="feat", bufs=6) as featp, \
             tc.tile_pool(name="oh", bufs=3) as ohp, \
             tc.tile_pool(name="rel", bufs=3) as relp, \
             tc.tile_pool(name="stage", bufs=3) as stagep, \
             tc.tile_pool(name="psum", bufs=8, space="PSUM") as psump:
            zeros_t = constp.tile([P, HCOL * C_OUT], bf)
            if SIM_SAFE:
                nc.vector.memset(zeros_t[:], 0.0)

            # rel arrives in per-oh-group pieces (53KB) so it never delays
            # a ~1MB feat group on either HWDGE ring; piece k is prefetched
            # one oh-group ahead of its use.
            rel_tiles = {}

            def rel_piece(k):
                if k in rel_tiles or k >= len(oh_starts):
                    return
                g0 = oh_starts[k]
                g1 = (oh_starts[k + 1] if k + 1 < len(oh_starts) else nchunk)
                rt = relp.tile([P, OH_G], bf, name="relt")
                eng = nc.sync if k % 2 == 0 else nc.scalar
                eng.dma_start(out=rt[:, :g1 - g0], in_=rel[:, g0:g1])
                rel_tiles[k] = rt


            bank_tile = {}
            fill_tile = {}
            f_t = oh_t = None
            ft0 = oh0 = 0
            fidx = oidx = 0
            ncopy = 0
            for q in range(nquad):
                strips = [4 * q + d for d in range(4) if 4 * q + d < nstrip]
                t0q = 16 * q

                if fidx < len(feat_starts) and feat_starts[fidx] == t0q:
                    g0 = feat_starts[fidx]
                    g1 = (feat_starts[fidx + 1]
                          if fidx + 1 < len(feat_starts) else nchunk)
                    ng = min(g1, nchunk) - g0
                    f_t = featp.tile([P, GG, C_OUT], fdt, name="ft")
                    # alternate the two HWDGE rings so consecutive group
                    # DMAs overlap instead of serializing FIFO on one ring
                    deng = nc.sync if fidx % 2 == 0 else nc.scalar
                    deng.dma_start(
                        out=f_t[:, :ng],
                        in_=pts[:, g0 * C_OUT:(g0 + ng) * C_OUT].rearrange(
                            "p (t d) -> p t d", d=C_OUT))
                    ft0 = g0
                    fidx += 1
                if oidx < len(oh_starts) and oh_starts[oidx] == t0q:
                    rel_piece(oidx)
                    rel_piece(oidx + 1)  # prefetch the next piece
                    g0 = oh_starts[oidx]
                    g1 = (oh_starts[oidx + 1]
                          if oidx + 1 < len(oh_starts) else nchunk)
                    ng = min(g1, nchunk) - g0
                    oh_t = ohp.tile([P, wgen, OH_G], bf, name="oh")
                    rsrc = rel_tiles[oidx][:, :ng]
                    for s in range(wgen):
                        nc.vector.tensor_scalar(
                            out=oh_t[:, s, :ng],
                            in0=rsrc,
                            scalar1=float(s), scalar2=None,
                            op0=mybir.AluOpType.is_equal)
                    oh0 = g0
                    oidx += 1

                geoms = {}
                for i in strips:
                    geoms[i] = strip_geom(i)
                    f, b, v, h = geoms[i]
                    if (f, b) not in bank_tile:
                        ps = psump.tile([P, HCOL * C_OUT], mybir.dt.float32,
                                        name="psbank")
                        bank_tile[(f, b)] = ps
                        if SIM_SAFE:
                            # zero-fill the bank (zero weights x anything) so
                            # rows/cols no strip writes are defined
                            w = fill_w[f]
                            nc.tensor.matmul(
                                out=ps[:, :w], lhsT=zeros_t[:, :P],
                                rhs=zeros_t[:, :w], start=True, stop=True,
                                skip_group_check=True)
                    if f not in fill_tile:
                        fill_tile[f] = stagep.tile(
                            [P, NB, HCOL * C_OUT], bf, name="fillst")
                # consecutive strips land in different PE col-groups (the
                # diagonal v assignment) -> LDW pull-ahead at strip edges
                for i in strips:
                    f, b, v, h = geoms[i]
                    for c in range(SPC):
                        t = SPC * i + c
                        lhsT = oh_t[:, :, t - oh0]
                        nc.tensor.matmul(
                            out=bank_tile[(f, b)][
                                32 * v:32 * v + wgen,
                                C_OUT * h:C_OUT * (h + 1)],
                            lhsT=lhsT,
                            rhs=f_t[:, t - ft0],
                            start=(c == 0),
                            stop=(c == SPC - 1),
                            tile_position=(0, 32 * v),
                            skip_group_check=True,
                        )
                for i in strips:
                    f, b, v, h = geoms[i]
                    w = fill_w[f]
                    if bank_last[(f, b)] == i:
                        st = fill_tile[f]
                        if ncopy % 2 == 0:
                            nc.scalar.copy(out=st[:, b, :w],
                                           in_=bank_tile[(f, b)][:, :w])
                        else:
                            nc.vector.tensor_copy(
                                out=st[:, b, :w],
                                in_=bank_tile[(f, b)][:, :w])
                        ncopy += 1
                    if fill_last[f] == i:
                        deng = nc.sync if f % 2 == 0 else nc.scalar
                        deng.dma_start(
                            out=wout[f * NB:(f + 1) * NB, :, :w].rearrange(
                                "b p c -> p b c"),
                            in_=fill_tile[f][:, :, :w])

    nc.compile()
    _COMPILED[key] = nc
    return nc


def _run_on_hw(nc, in_maps, trace=False):
    from concourse.bass_utils import run_bass_kernel_spmd
    from concourse.bass_interp import get_hw_module

    if trace:
        try:
            import ntff_hook
            ntff_hook.install()
        except Exception:
            pass
    hw_m = get_hw_module(nc.m)
    old_m = nc.m
    nc.m = hw_m
    try:
        res = run_bass_kernel_spmd(
            nc, in_maps, core_ids=list(range(NCORES)), trace=trace,
        )
    finally:
        nc.m = old_m
    return res


def kernel(cam_feats, camera_intrinsics, camera2lidar, img_aug_matrix,
           lidar_aug_matrix, _trace=False, _return_results=False):
    cam = np.ascontiguousarray(np.asarray(cam_feats, np.float32))
    Npts = cam.size // C_OUT
    fnp = ml_dtypes.float8_e3m4 if USE_FP8 else ml_dtypes.bfloat16
    cam_q = cam.reshape(Npts, C_OUT).astype(fnp)

    seg, kept = _segments(camera_intrinsics, camera2lidar,
                          img_aug_matrix, lidar_aug_matrix)
    plan = _plan(seg, kept)
    nchunk, nstrip = plan['nchunk'], plan['nstrip']

    in_maps = []
    for k in range(NCORES):
        shard = cam_q[plan['rows'][k].reshape(-1)]
        shard = shard.reshape(nchunk, P, C_OUT).transpose(1, 0, 2)
        shard = np.ascontiguousarray(shard).reshape(P, nchunk * C_OUT)
        relk = np.ascontiguousarray(
            plan['rel'][k].T.astype(np.float32)).astype(ml_dtypes.bfloat16)
        in_maps.append(dict(pts=shard, rel=relk))

    wgen = WGEN if plan['maxspan'] <= WGEN else WIN
    nc = _build_program(nchunk, nstrip, wgen)
    res = _run_on_hw(nc, in_maps, trace=_trace)

    # ---------------- host assembly ----------------
    vals = np.stack([np.asarray(r['wout']).astype(np.float32)
                     for r in res.results])  # [NCORES, nfb, 128, 480]

    # strip i -> wout[f*NB + b][32v:32v+32, 80h:80(h+1)]
    i_all = np.arange(nstrip)
    f_i, r_i = np.divmod(i_all, NB * SPB)
    b_i = r_i % NB
    j_i = r_i // NB
    v_i = (j_i + b_i) % 4
    h_i = j_i // 4
    fb = f_i * NB + b_i
    # gather each strip's [WIN, C_OUT] block
    rows = (32 * v_i)[:, None] + np.arange(WIN)[None, :]      # [nstrip, WIN]
    cols = (C_OUT * h_i)[:, None] + np.arange(C_OUT)[None, :]  # [nstrip, C_OUT]
    strip_vals = vals[:, fb[:, None, None], rows[:, :, None], cols[:, None, :]]
    # -> [NCORES, nstrip, WIN, C_OUT]

    segs = plan['slot_seg']
    valid = (np.arange(WIN)[None, None, :] < plan['span'][:, :, None])
    s_all = segs.reshape(NCORES, -1)[valid.reshape(NCORES, -1)]
    v_all = strip_vals.reshape(NCORES, nstrip * WIN, C_OUT)[
        valid.reshape(NCORES, -1)]
    o2 = np.argsort(s_all, kind='stable')
    s2 = s_all[o2]
    v2 = v_all[o2]
    acc = np.zeros((NSEG, C_OUT), np.float32)
    if len(s2):
        starts = np.r_[0, np.flatnonzero(np.diff(s2)) + 1]
        sums = np.add.reduceat(v2, starts, axis=0)
        useg = s2[starts]
        acc[useg] = sums / np.maximum(plan['counts'][useg], 1)[:, None]

    out = acc.reshape(NX[2], NX[0], NX[1], C_OUT).transpose(0, 3, 1, 2)
    out = out.reshape(1, NX[2] * C_OUT, NX[0], NX[1]).astype(np.float32)
    if _return_results:
        return out, res
    return out
